# revision 31
# baseline (speedup 1.0000x reference)
"""Multi-head attention kernel for Trainium2, 8 NeuronCores.

Problem: B=4, S=2048, HID=1024, H=16 heads, D=64.
  Q = q@Wq, K = k@Wk, V = v@Wv (reshaped to heads)
  O = softmax(Q K^T / sqrt(D)) V ;  out = O @ Wo

Sharding (hardcoded): core c handles batch b=c//2 and head-half hf=c%2
(8 of 16 heads via column-parallel Wq/Wk/Wv, row-parallel Wo).  Each core
returns a partial output [S, HID]; the host sums the two head-halves per
batch.

v5 design (single fused pipeline, ACT-exp paced):
  - Host pre-transposes q/k/v to [HID, S] and converts x + weights to
    bf16: no on-chip transposes; every projection matmul reads xT with
    the contraction dim on partitions.
  - The softmax exp on the scalar (ACT) engine is the hard floor
    (8 heads x 2048 x 2048 = 33.5M elem/core at ~1.06us per 1024-wide
    drain ~= 272us).  The kernel is one flat stream of 256 k-tile steps
    (16 rounds x 16 k-tiles) of [scores pair -> exp -> attn@V lagging 3
    steps], with K/Q/Y projection matmuls drip-fed ~2 per step from a
    global generator deque.  The fillers both hide the projection work
    inside the ACT-paced slack and keep the PE dense enough that the
    HAM clock gate stays at full rate.
  - DMA descriptor generation costs ~0.6us per contiguous line on the
    issuing engine queue, so x tensors use 8 one-line dma_starts each,
    spread across the sync/scalar/gpsimd queues to issue in parallel.
  - Normalization per round r is emitted 2 steps into round r+1 (right
    after attn@V(r,15)): ovst copies first (releases the
    single-buffered ov psum), packed reciprocal ([33,512], rows 0/32),
    gpsimd partition-broadcast, multiply.  Y(qc) fillers are placed so
    their hp3 matmul trails the qc's last norm by >=4 k-tiles.
"""

import threading

import numpy as np

import concourse.bacc as bacc
import concourse.mybir as mybir
import concourse.tile as tile
from concourse.bass_utils import run_bass_kernel_spmd

DT = mybir.dt
AF = mybir.ActivationFunctionType

B, S, HID, H = 4, 2048, 1024, 16
D = HID // H               # 64
E = 512                    # local hidden (8 heads)
HLOC = 8                   # heads per core
NHP = 4                    # head pairs per core
SC = 4                     # s-chunks of 512
ST = 16                    # s-tiles of 128
CB = 8                     # contraction blocks of 128 (over HID)
ET = 4                     # e-tiles of 128 in Q^T/K^T
QCW = 512                  # q-chunk width
VW = D + 2                 # V row width: 64 data + ones col + pad (4B align)
LAG = 4                    # attn@V trails scores/exp by this many k-tiles
SCALE = 1.0 / np.sqrt(np.float32(D))   # 0.125

_lock = threading.Lock()
_cache = {}


def _build():
    nc = bacc.Bacc(None)
    xqT = nc.declare_dram_parameter("xqT", [HID, S], DT.bfloat16, isOutput=False)
    xkT = nc.declare_dram_parameter("xkT", [HID, S], DT.bfloat16, isOutput=False)
    xvT = nc.declare_dram_parameter("xvT", [HID, S], DT.bfloat16, isOutput=False)
    wq = nc.declare_dram_parameter("wq", [HID, E], DT.bfloat16, isOutput=False)
    wk = nc.declare_dram_parameter("wk", [HID, E], DT.bfloat16, isOutput=False)
    wv = nc.declare_dram_parameter("wv", [HID, E], DT.bfloat16, isOutput=False)
    wo = nc.declare_dram_parameter("wo", [E, HID], DT.bfloat16, isOutput=False)
    y = nc.declare_dram_parameter("y", [S, HID], DT.float32, isOutput=True)

    with tile.TileContext(nc) as tc:
        with (
            tc.tile_pool(name="wpool", bufs=1) as wpool,
            tc.tile_pool(name="xpool", bufs=1) as xpool,
            tc.tile_pool(name="xqpool", bufs=1) as xqpool,
            tc.tile_pool(name="qkv", bufs=1) as qkvp,
            tc.tile_pool(name="pt", bufs=LAG + 2) as ptp,
            tc.tile_pool(name="norm2", bufs=2) as norm2p,
            tc.tile_pool(name="yout", bufs=2) as youtp,
            tc.tile_pool(name="ps_proj", bufs=2, space="PSUM") as ps_proj,
            tc.tile_pool(name="ps_s", bufs=2, space="PSUM") as ps_s,
            tc.tile_pool(name="ps_ov", bufs=1, space="PSUM") as ps_ov,
        ):
            # ---- DMAs: descriptor-gen spread across engine queues so
            # transfers overlap; x tensors in (cb, s-half) chunks so the
            # V/K projections start as soon as their half has landed.
            wv_sb = wpool.tile([128, CB, E], DT.bfloat16, tag="wv")
            nc.sync.dma_start(
                out=wv_sb, in_=wv.rearrange("(cb p) e -> p cb e", p=128))
            wk_sb = wpool.tile([128, CB, E], DT.bfloat16, tag="wk")
            nc.scalar.dma_start(
                out=wk_sb, in_=wk.rearrange("(cb p) e -> p cb e", p=128))
            xv_sb = xpool.tile([128, CB, S], DT.bfloat16, tag="xv")
            xk_sb = xpool.tile([128, CB, S], DT.bfloat16, tag="xk")
            for h in range(2):
                hs = slice(h * (S // 2), (h + 1) * (S // 2))
                for cb in range(CB):
                    nc.sync.dma_start(
                        out=xv_sb[:, cb, hs],
                        in_=xvT[cb * 128:(cb + 1) * 128, hs])
            for h in range(2):
                hs = slice(h * (S // 2), (h + 1) * (S // 2))
                for cb in range(CB):
                    nc.sync.dma_start(
                        out=xk_sb[:, cb, hs],
                        in_=xkT[cb * 128:(cb + 1) * 128, hs])
            wq_sb = wpool.tile([128, CB, E], DT.bfloat16, tag="wq")
            nc.scalar.dma_start(
                out=wq_sb, in_=wq.rearrange("(cb p) e -> p cb e", p=128))

            xq_tiles = {}

            def q_prep(qc, eng=None):
                xq_sb = xqpool.tile([128, CB, QCW], DT.bfloat16, tag="xq",
                                    name=f"xq{qc}")
                (eng or nc.sync).dma_start(
                    out=xq_sb,
                    in_=xqT[:, qc * QCW:(qc + 1) * QCW].rearrange(
                        "(cb p) s -> p cb s", p=128),
                )
                xq_tiles[qc] = xq_sb

            q_prep(0, eng=nc.scalar)
            wo_sb = wpool.tile([128, NHP, HID], DT.bfloat16, tag="wo")
            nc.scalar.dma_start(
                out=wo_sb, in_=wo.rearrange("(eb p) n -> p eb n", p=128))

            # ---- persistent SBUF tensors ----
            qT = qkvp.tile([128, ET, S], DT.bfloat16, tag="qT")
            kT = qkvp.tile([128, ET, S], DT.bfloat16, tag="kT")
            v_sb = qkvp.tile([128, ST, HLOC, VW], DT.bfloat16, tag="v")
            vpad = qkvp.tile([128, 64], DT.bfloat16, tag="vpad")
            _ = vpad
            nc.vector.memset(v_sb[:, :, :, D:D + 1], 1.0)
            v_flat = v_sb.rearrange("p a h w -> p (a h w)")
            ot_sb = qkvp.tile([128, NHP, S], DT.bfloat16, tag="ot")

            # preload the exp table set (~2.7us) during the head
            warm = norm2p.tile([1, 8], DT.float32, tag="warm")
            nc.vector.memset(warm, 0.0)
            nc.scalar.activation(out=warm, in_=warm, func=AF.Exp)

            # ---- projection units (generators: ~2 matmuls per step) ----
            def v_unit(st):
                pp = ps_proj.tile([128, E], DT.float32, tag="pp", name=f"vp{st}")
                for cb in range(CB):
                    nc.tensor.matmul(
                        pp,
                        xv_sb[:, cb, st * 128:(st + 1) * 128],
                        wv_sb[:, cb, :],
                        start=(cb == 0),
                        stop=(cb == CB - 1),
                    )
                nc.vector.tensor_copy(
                    v_sb[:, st, :, 0:D],
                    pp.rearrange("p (h d) -> p h d", h=HLOC),
                )

            def k_unit(et, sc):
                pp = ps_proj.tile([128, QCW], DT.float32, tag="pp",
                                  name=f"kp{et}_{sc}")
                for cb in range(CB):
                    nc.tensor.matmul(
                        pp,
                        wk_sb[:, cb, et * 128:(et + 1) * 128],
                        xk_sb[:, cb, sc * QCW:(sc + 1) * QCW],
                        start=(cb == 0),
                        stop=(cb == CB - 1),
                    )
                    if cb % 2 == 1 and cb < CB - 1:
                        yield
                nc.vector.tensor_copy(kT[:, et, sc * QCW:(sc + 1) * QCW], pp)

            def q_unit(qc, et):
                xq_sb = xq_tiles[qc]
                pp = ps_proj.tile([128, QCW], DT.float32, tag="pp",
                                  name=f"qp{qc}_{et}")
                for cb in range(CB):
                    nc.tensor.matmul(
                        pp,
                        wq_sb[:, cb, et * 128:(et + 1) * 128],
                        xq_sb[:, cb, :],
                        start=(cb == 0),
                        stop=(cb == CB - 1),
                    )
                    if cb % 2 == 1 and cb < CB - 1:
                        yield
                nc.vector.tensor_copy(qT[:, et, qc * QCW:(qc + 1) * QCW], pp)

            y_tiles = {}
            yp_tiles = {}

            def y_start(st, nch, n_hp):
                if nch == 0:
                    y_tiles[st] = youtp.tile(
                        [128, HID], DT.float32, tag="y", name=f"ysb{st}")
                yp = ps_proj.tile([128, QCW], DT.float32, tag="pp",
                                  name=f"yp{st}_{nch}")
                yp_tiles[(st, nch)] = yp
                for hp in range(n_hp):
                    nc.tensor.matmul(
                        yp,
                        ot_sb[:, hp, st * 128:(st + 1) * 128],
                        wo_sb[:, hp, nch * QCW:(nch + 1) * QCW],
                        start=(hp == 0),
                        stop=False,
                    )
                    if hp == 1:
                        yield

            def y_finish(st, nch, hp0):
                yp = yp_tiles.pop((st, nch))
                ysb = y_tiles[st]
                for hp in range(hp0, NHP):
                    nc.tensor.matmul(
                        yp,
                        ot_sb[:, hp, st * 128:(st + 1) * 128],
                        wo_sb[:, hp, nch * QCW:(nch + 1) * QCW],
                        start=False,
                        stop=(hp == NHP - 1),
                    )
                nc.vector.tensor_copy(ysb[:, nch * QCW:(nch + 1) * QCW], yp)
                if nch == 1:
                    nc.sync.dma_start(out=y[st * 128:(st + 1) * 128, :], in_=ysb)

            def y_unit(st, nch):
                yield from y_start(st, nch, 2)
                y_finish(st, nch, 2)

            def y_prefix(st, nch):
                yield from y_start(st, nch, 2)

            def dma_unit(fn):
                fn()
                return
                yield  # generator

            def skip(n):
                for _ in range(n):
                    yield

            junk_n = [0]

            def junk_unit(n):
                # dead score-shaped matmuls: keep the PE dense enough that
                # the HAM clock gate stays at full rate in thin rounds
                for i in range(n):
                    junk_n[0] += 1
                    jp = ps_s.tile([128, 2, QCW], DT.float32, tag="ps_s",
                                   name=f"junk{junk_n[0]}")
                    for par in range(2):
                        nc.tensor.matmul(
                            jp[:, par, :],
                            kT[par * D:par * D + D, 0, 0:128],
                            qT[par * D:par * D + D, 0, 0:QCW],
                            start=True,
                            stop=True,
                        )
                    yield

            # ---- serial head: V, K et0, Q qc0 et0..1 ----
            for st in range(ST):
                v_unit(st)
            for sc in range(SC):
                for _ in k_unit(0, sc):
                    pass
            for et in range(2):
                for _ in q_unit(0, et):
                    pass

            # ---- filler schedule (deque order; ~1 step per k-tile).
            # K(et,sc) before round hp=et reaches k-tile 4*sc (JIT);
            # Q(qc,et) before round 4qc+et; Y(qc,..) with hp3 >=4 k-tiles
            # after norm(qc,hp3) which is emitted at round 4qc+4, kt2.
            fillers = {
                0: [dma_unit(lambda: q_prep(1)), k_unit(1, 0), k_unit(1, 1), k_unit(1, 2)],
                1: [k_unit(1, 3), q_unit(0, 2), k_unit(2, 0), k_unit(2, 1)],
                2: [k_unit(2, 2), k_unit(2, 3), k_unit(3, 0), q_unit(0, 3)],
                3: [k_unit(3, 1), k_unit(3, 2), k_unit(3, 3), q_unit(1, 0)],
                4: [q_unit(1, 1), q_unit(1, 2), q_unit(1, 3),
                    dma_unit(lambda: q_prep(2)), junk_unit(2)],
                5: [y_unit(0, 0), y_unit(0, 1), y_unit(1, 0), q_unit(2, 0)],
                6: [y_unit(1, 1), y_unit(2, 0), y_unit(2, 1), q_unit(2, 1)],
                7: [y_unit(3, 0), y_unit(3, 1), q_unit(2, 2), q_unit(2, 3),
                    dma_unit(lambda: q_prep(3))],
                8: [q_unit(3, 0), junk_unit(6)],
                9: [y_unit(4, 0), y_unit(4, 1), y_unit(5, 0), q_unit(3, 1)],
                10: [y_unit(5, 1), y_unit(6, 0), y_unit(6, 1), q_unit(3, 2)],
                11: [y_unit(7, 0), y_unit(7, 1), q_unit(3, 3), junk_unit(3)],
                12: [junk_unit(10)],
                13: [y_unit(8, 0), y_unit(8, 1), y_unit(9, 0), y_unit(9, 1),
                     junk_unit(2)],
                14: [y_unit(10, 0), y_unit(10, 1), y_unit(11, 0), y_unit(11, 1),
                     junk_unit(2)],
                15: [junk_unit(7)],
            }
            pending = []

            # ---- attention: flat stream of 256 k-tile steps ----
            rounds = [(qc, hp) for qc in range(SC) for hp in range(NHP)]
            ov_tiles = {}
            ptq = []  # (r, kt, ptile)

            def attn_v(r, kt, ptile):
                qc, hp = rounds[r]
                if r not in ov_tiles:
                    ov_tiles[r] = [
                        ps_ov.tile([128, QCW], DT.float32, tag=f"ov{par}",
                                   name=f"ov{r}_{par}")
                        for par in range(2)
                    ]
                ovs = ov_tiles[r]
                for par in range(2):
                    v0 = (kt * HLOC + 2 * hp + par) * VW
                    w = 128 if v0 + 128 <= ST * HLOC * VW else D + 1
                    nc.tensor.matmul(
                        ovs[par][0:w, :],
                        v_flat[:, v0:v0 + w],
                        ptile[:, par, :],
                        start=(kt == 0),
                        stop=(kt == ST - 1),
                    )

            norm_state = {}

            def norm_a(r):
                # copies only: releases the ov psum banks quickly and
                # keeps the DVE FIFO clear for filler drains
                ovs = ov_tiles.pop(r)
                ovsts = []
                for par in range(2):
                    ovst = norm2p.tile([D, QCW], DT.float32, tag=f"ovst{par}")
                    nc.vector.tensor_copy(ovst, ovs[par][0:D, :])
                    ovsts.append(ovst)
                sums = norm2p.tile([33, QCW], DT.float32, tag="sums")
                for par in range(2):
                    nc.vector.tensor_copy(
                        sums[32 * par:32 * par + 1, :], ovs[par][D:D + 1, :])
                norm_state[r] = (ovsts, sums)

            def norm_b(r):
                qc, hp = rounds[r]
                q0 = qc * QCW
                ovsts, sums = norm_state.pop(r)
                nc.vector.reciprocal(sums, sums)
                for par in range(2):
                    if par == 1:
                        nc.vector.tensor_copy(sums[0:1, :], sums[32:33, :])
                    bc = norm2p.tile([D, QCW], DT.float32, tag="bc")
                    nc.gpsimd.partition_broadcast(bc, sums[0:1, :])
                    nc.vector.tensor_mul(
                        ot_sb[par * D:par * D + D, hp, q0:q0 + QCW],
                        ovsts[par],
                        bc,
                    )

            for i in range(ST * len(rounds)):
                r, kt = divmod(i, ST)
                qc, hp = rounds[r]
                if kt == 0:
                    pending.extend(fillers.get(r, ()))
                sps = ps_s.tile([128, 2, QCW], DT.float32, tag="ps_s")
                for par in range(2):
                    prow = slice(par * D, par * D + D)
                    nc.tensor.matmul(
                        sps[:, par, :],
                        kT[prow, hp, kt * 128:(kt + 1) * 128],
                        qT[prow, hp, qc * QCW:(qc + 1) * QCW],
                        start=True,
                        stop=True,
                    )
                ptile = ptp.tile([128, 2, QCW], DT.bfloat16, tag="pt")
                nc.scalar.activation(
                    out=ptile, in_=sps, func=AF.Exp, scale=float(SCALE),
                )
                ptq.append((r, kt, ptile))
                if len(ptq) > LAG:
                    attn_v(*ptq.pop(0))
                if kt == LAG - 1 and r > 0:
                    norm_a(r - 1)
                if kt == LAG + 3 and r > 0:
                    norm_b(r - 1)
                # drip-feed ~2 filler matmuls into the PE slack
                if pending:
                    try:
                        next(pending[0])
                    except StopIteration:
                        pending.pop(0)

            while ptq:
                attn_v(*ptq.pop(0))
            for g in pending:
                for _ in g:
                    pass
            norm_a(15)
            # keep-warm bridge spanning the final norm chain: emitted
            # BEFORE the Y matmuls (whose weight-loads serialize on the
            # last norm multiply) so the PE queue isn't head-of-line
            # blocked and the HAM clock gate stays warm
            for _ in junk_unit(22):
                pass
            for nch in range(2):
                for _ in y_start(12, nch, 3):
                    pass
            norm_b(15)

            # ---- tail: finish Y of the last q-chunk ----
            y_finish(12, 0, 3)
            y_finish(12, 1, 3)
            for st in range(13, ST):
                for nch in range(2):
                    for _ in y_unit(st, nch):
                        pass

    nc.finalize()
    return nc


def _get_nc():
    with _lock:
        if "nc" not in _cache:
            _cache["nc"] = _build()
        return _cache["nc"]


def _in_maps(q, k, v, Wq, Wk, Wv, Wo):
    import ml_dtypes

    bf16 = ml_dtypes.bfloat16
    xT = {}
    for b in range(B):
        xT[b] = tuple(
            np.ascontiguousarray(t[b].astype(bf16).T) for t in (q, k, v)
        )
    w_bf = [
        (np.ascontiguousarray(W[:, hf * E:(hf + 1) * E].astype(bf16)) if W is not Wo
         else np.ascontiguousarray(W[hf * E:(hf + 1) * E, :].astype(bf16)))
        for hf in range(2) for W in (Wq, Wk, Wv, Wo)
    ]
    maps = []
    for c in range(8):
        b, hf = c // 2, c % 2
        qt, kt, vt = xT[b]
        wqc, wkc, wvc, woc = w_bf[hf * 4:(hf + 1) * 4]
        maps.append({
            "xqT": qt,
            "xkT": kt,
            "xvT": vt,
            "wq": wqc,
            "wk": wkc,
            "wv": wvc,
            "wo": woc,
        })
    return maps


def run(q, k, v, Wq, Wk, Wv, Wo, **spmd_kwargs):
    nc = _get_nc()
    res = run_bass_kernel_spmd(
        nc, _in_maps(q, k, v, Wq, Wk, Wv, Wo), core_ids=list(range(8)),
        **spmd_kwargs,
    )
    out = np.empty((B, S, HID), dtype=np.float32)
    for b in range(B):
        out[b] = res.results[2 * b]["y"] + res.results[2 * b + 1]["y"]
    return out, res


def kernel(q, k, v, Wq, Wk, Wv, Wo):
    out, _ = run(q, k, v, Wq, Wk, Wv, Wo)
    return out


# revision 32
# speedup vs baseline: 1.0097x; 1.0097x over previous
"""Multi-head attention kernel for Trainium2, 8 NeuronCores.

Problem: B=4, S=2048, HID=1024, H=16 heads, D=64.
  Q = q@Wq, K = k@Wk, V = v@Wv (reshaped to heads)
  O = softmax(Q K^T / sqrt(D)) V ;  out = O @ Wo

Sharding (hardcoded): core c handles batch b=c//2 and head-half hf=c%2
(8 of 16 heads via column-parallel Wq/Wk/Wv, row-parallel Wo).  Each core
returns a partial output [S, HID]; the host sums the two head-halves per
batch.

v5 design (single fused pipeline, ACT-exp paced):
  - Host pre-transposes q/k/v to [HID, S] and converts x + weights to
    bf16: no on-chip transposes; every projection matmul reads xT with
    the contraction dim on partitions.
  - The softmax exp on the scalar (ACT) engine is the hard floor
    (8 heads x 2048 x 2048 = 33.5M elem/core at ~1.06us per 1024-wide
    drain ~= 272us).  The kernel is one flat stream of 256 k-tile steps
    (16 rounds x 16 k-tiles) of [scores pair -> exp -> attn@V lagging 3
    steps], with K/Q/Y projection matmuls drip-fed ~2 per step from a
    global generator deque.  The fillers both hide the projection work
    inside the ACT-paced slack and keep the PE dense enough that the
    HAM clock gate stays at full rate.
  - DMA descriptor generation costs ~0.6us per contiguous line on the
    issuing engine queue, so x tensors use 8 one-line dma_starts each,
    spread across the sync/scalar/gpsimd queues to issue in parallel.
  - Normalization per round r is emitted 2 steps into round r+1 (right
    after attn@V(r,15)): ovst copies first (releases the
    single-buffered ov psum), packed reciprocal ([33,512], rows 0/32),
    gpsimd partition-broadcast, multiply.  Y(qc) fillers are placed so
    their hp3 matmul trails the qc's last norm by >=4 k-tiles.
"""

import threading

import numpy as np

import concourse.bacc as bacc
import concourse.mybir as mybir
import concourse.tile as tile
from concourse.bass_utils import run_bass_kernel_spmd

DT = mybir.dt
AF = mybir.ActivationFunctionType

B, S, HID, H = 4, 2048, 1024, 16
D = HID // H               # 64
E = 512                    # local hidden (8 heads)
HLOC = 8                   # heads per core
NHP = 4                    # head pairs per core
SC = 4                     # s-chunks of 512
ST = 16                    # s-tiles of 128
CB = 8                     # contraction blocks of 128 (over HID)
ET = 4                     # e-tiles of 128 in Q^T/K^T
QCW = 512                  # q-chunk width
VW = D + 2                 # V row width: 64 data + ones col + pad (4B align)
LAG = 4                    # attn@V trails scores/exp by this many k-tiles
SCALE = 1.0 / np.sqrt(np.float32(D))   # 0.125

_lock = threading.Lock()
_cache = {}


def _build():
    nc = bacc.Bacc(None)
    xqT = nc.declare_dram_parameter("xqT", [HID, S], DT.bfloat16, isOutput=False)
    xkT = nc.declare_dram_parameter("xkT", [HID, S], DT.bfloat16, isOutput=False)
    xvT = nc.declare_dram_parameter("xvT", [HID, S], DT.bfloat16, isOutput=False)
    wq = nc.declare_dram_parameter("wq", [HID, E], DT.bfloat16, isOutput=False)
    wk = nc.declare_dram_parameter("wk", [HID, E], DT.bfloat16, isOutput=False)
    wv = nc.declare_dram_parameter("wv", [HID, E], DT.bfloat16, isOutput=False)
    wo = nc.declare_dram_parameter("wo", [E, HID], DT.bfloat16, isOutput=False)
    y = nc.declare_dram_parameter("y", [S, HID], DT.float32, isOutput=True)

    with tile.TileContext(nc) as tc:
        with (
            tc.tile_pool(name="wpool", bufs=1) as wpool,
            tc.tile_pool(name="xpool", bufs=1) as xpool,
            tc.tile_pool(name="xqpool", bufs=1) as xqpool,
            tc.tile_pool(name="qkv", bufs=1) as qkvp,
            tc.tile_pool(name="pt", bufs=LAG + 2) as ptp,
            tc.tile_pool(name="norm2", bufs=2) as norm2p,
            tc.tile_pool(name="yout", bufs=2) as youtp,
            tc.tile_pool(name="ps_proj", bufs=2, space="PSUM") as ps_proj,
            tc.tile_pool(name="ps_s", bufs=2, space="PSUM") as ps_s,
            tc.tile_pool(name="ps_ov", bufs=1, space="PSUM") as ps_ov,
        ):
            # ---- DMAs: descriptor-gen spread across engine queues so
            # transfers overlap; x tensors in (cb, s-half) chunks so the
            # V/K projections start as soon as their half has landed.
            wv_sb = wpool.tile([128, CB, E], DT.bfloat16, tag="wv")
            nc.sync.dma_start(
                out=wv_sb, in_=wv.rearrange("(cb p) e -> p cb e", p=128))
            wk_sb = wpool.tile([128, CB, E], DT.bfloat16, tag="wk")
            nc.scalar.dma_start(
                out=wk_sb, in_=wk.rearrange("(cb p) e -> p cb e", p=128))
            xv_sb = xpool.tile([128, CB, S], DT.bfloat16, tag="xv")
            xk_sb = xpool.tile([128, CB, S], DT.bfloat16, tag="xk")
            for h in range(2):
                hs = slice(h * (S // 2), (h + 1) * (S // 2))
                for cb in range(CB):
                    nc.sync.dma_start(
                        out=xv_sb[:, cb, hs],
                        in_=xvT[cb * 128:(cb + 1) * 128, hs])
                for cb in range(CB):
                    nc.scalar.dma_start(
                        out=xk_sb[:, cb, hs],
                        in_=xkT[cb * 128:(cb + 1) * 128, hs])
            wq_sb = wpool.tile([128, CB, E], DT.bfloat16, tag="wq")
            nc.scalar.dma_start(
                out=wq_sb, in_=wq.rearrange("(cb p) e -> p cb e", p=128))

            xq_tiles = {}

            def q_prep(qc, eng=None):
                xq_sb = xqpool.tile([128, CB, QCW], DT.bfloat16, tag="xq",
                                    name=f"xq{qc}")
                (eng or nc.sync).dma_start(
                    out=xq_sb,
                    in_=xqT[:, qc * QCW:(qc + 1) * QCW].rearrange(
                        "(cb p) s -> p cb s", p=128),
                )
                xq_tiles[qc] = xq_sb

            q_prep(0, eng=nc.scalar)
            wo_sb = wpool.tile([128, NHP, HID], DT.bfloat16, tag="wo")
            nc.scalar.dma_start(
                out=wo_sb, in_=wo.rearrange("(eb p) n -> p eb n", p=128))

            # ---- persistent SBUF tensors ----
            qT = qkvp.tile([128, ET, S], DT.bfloat16, tag="qT")
            kT = qkvp.tile([128, ET, S], DT.bfloat16, tag="kT")
            v_sb = qkvp.tile([128, ST, HLOC, VW], DT.bfloat16, tag="v")
            vpad = qkvp.tile([128, 64], DT.bfloat16, tag="vpad")
            _ = vpad
            nc.vector.memset(v_sb[:, :, :, D:D + 1], 1.0)
            v_flat = v_sb.rearrange("p a h w -> p (a h w)")
            ot_sb = qkvp.tile([128, NHP, S], DT.bfloat16, tag="ot")

            # preload the exp table set (~2.7us) during the head
            warm = norm2p.tile([1, 8], DT.float32, tag="warm")
            nc.vector.memset(warm, 0.0)
            nc.scalar.activation(out=warm, in_=warm, func=AF.Exp)

            # ---- projection units (generators: ~2 matmuls per step) ----
            def v_unit(st):
                pp = ps_proj.tile([128, E], DT.float32, tag="pp", name=f"vp{st}")
                for cb in range(CB):
                    nc.tensor.matmul(
                        pp,
                        xv_sb[:, cb, st * 128:(st + 1) * 128],
                        wv_sb[:, cb, :],
                        start=(cb == 0),
                        stop=(cb == CB - 1),
                    )
                nc.vector.tensor_copy(
                    v_sb[:, st, :, 0:D],
                    pp.rearrange("p (h d) -> p h d", h=HLOC),
                )

            def k_unit(et, sc):
                pp = ps_proj.tile([128, QCW], DT.float32, tag="pp",
                                  name=f"kp{et}_{sc}")
                for cb in range(CB):
                    nc.tensor.matmul(
                        pp,
                        wk_sb[:, cb, et * 128:(et + 1) * 128],
                        xk_sb[:, cb, sc * QCW:(sc + 1) * QCW],
                        start=(cb == 0),
                        stop=(cb == CB - 1),
                    )
                    if cb % 2 == 1 and cb < CB - 1:
                        yield
                nc.vector.tensor_copy(kT[:, et, sc * QCW:(sc + 1) * QCW], pp)

            def q_unit(qc, et):
                xq_sb = xq_tiles[qc]
                pp = ps_proj.tile([128, QCW], DT.float32, tag="pp",
                                  name=f"qp{qc}_{et}")
                for cb in range(CB):
                    nc.tensor.matmul(
                        pp,
                        wq_sb[:, cb, et * 128:(et + 1) * 128],
                        xq_sb[:, cb, :],
                        start=(cb == 0),
                        stop=(cb == CB - 1),
                    )
                    if cb % 2 == 1 and cb < CB - 1:
                        yield
                nc.vector.tensor_copy(qT[:, et, qc * QCW:(qc + 1) * QCW], pp)

            y_tiles = {}
            yp_tiles = {}

            def y_start(st, nch, n_hp):
                if nch == 0:
                    y_tiles[st] = youtp.tile(
                        [128, HID], DT.float32, tag="y", name=f"ysb{st}")
                yp = ps_proj.tile([128, QCW], DT.float32, tag="pp",
                                  name=f"yp{st}_{nch}")
                yp_tiles[(st, nch)] = yp
                for hp in range(n_hp):
                    nc.tensor.matmul(
                        yp,
                        ot_sb[:, hp, st * 128:(st + 1) * 128],
                        wo_sb[:, hp, nch * QCW:(nch + 1) * QCW],
                        start=(hp == 0),
                        stop=False,
                    )
                    if hp == 1:
                        yield

            def y_finish(st, nch, hp0):
                yp = yp_tiles.pop((st, nch))
                ysb = y_tiles[st]
                for hp in range(hp0, NHP):
                    nc.tensor.matmul(
                        yp,
                        ot_sb[:, hp, st * 128:(st + 1) * 128],
                        wo_sb[:, hp, nch * QCW:(nch + 1) * QCW],
                        start=False,
                        stop=(hp == NHP - 1),
                    )
                nc.vector.tensor_copy(ysb[:, nch * QCW:(nch + 1) * QCW], yp)
                if nch == 1:
                    nc.sync.dma_start(out=y[st * 128:(st + 1) * 128, :], in_=ysb)

            def y_unit(st, nch):
                yield from y_start(st, nch, 2)
                y_finish(st, nch, 2)

            def y_prefix(st, nch):
                yield from y_start(st, nch, 2)

            def dma_unit(fn):
                fn()
                return
                yield  # generator

            def skip(n):
                for _ in range(n):
                    yield

            junk_n = [0]

            def junk_unit(n):
                # dead score-shaped matmuls: keep the PE dense enough that
                # the HAM clock gate stays at full rate in thin rounds
                for i in range(n):
                    junk_n[0] += 1
                    jp = ps_s.tile([128, 2, QCW], DT.float32, tag="ps_s",
                                   name=f"junk{junk_n[0]}")
                    for par in range(2):
                        nc.tensor.matmul(
                            jp[:, par, :],
                            kT[par * D:par * D + D, 0, 0:128],
                            qT[par * D:par * D + D, 0, 0:QCW],
                            start=True,
                            stop=True,
                        )
                    yield

            # ---- serial head: V, K et0, Q qc0 et0..1 ----
            for st in range(ST):
                v_unit(st)
            for sc in range(SC):
                for _ in k_unit(0, sc):
                    pass
            for et in range(2):
                for _ in q_unit(0, et):
                    pass

            # ---- filler schedule (deque order; ~1 step per k-tile).
            # K(et,sc) before round hp=et reaches k-tile 4*sc (JIT);
            # Q(qc,et) before round 4qc+et; Y(qc,..) with hp3 >=4 k-tiles
            # after norm(qc,hp3) which is emitted at round 4qc+4, kt2.
            fillers = {
                0: [dma_unit(lambda: q_prep(1)), k_unit(1, 0), k_unit(1, 1), k_unit(1, 2)],
                1: [k_unit(1, 3), q_unit(0, 2), k_unit(2, 0), k_unit(2, 1)],
                2: [k_unit(2, 2), k_unit(2, 3), k_unit(3, 0), q_unit(0, 3)],
                3: [k_unit(3, 1), k_unit(3, 2), k_unit(3, 3), q_unit(1, 0)],
                4: [q_unit(1, 1), q_unit(1, 2), q_unit(1, 3),
                    dma_unit(lambda: q_prep(2)), junk_unit(2)],
                5: [y_unit(0, 0), y_unit(0, 1), y_unit(1, 0), q_unit(2, 0)],
                6: [y_unit(1, 1), y_unit(2, 0), y_unit(2, 1), q_unit(2, 1)],
                7: [y_unit(3, 0), y_unit(3, 1), q_unit(2, 2), q_unit(2, 3),
                    dma_unit(lambda: q_prep(3))],
                8: [q_unit(3, 0), junk_unit(6)],
                9: [y_unit(4, 0), y_unit(4, 1), y_unit(5, 0), q_unit(3, 1)],
                10: [y_unit(5, 1), y_unit(6, 0), y_unit(6, 1), q_unit(3, 2)],
                11: [y_unit(7, 0), y_unit(7, 1), q_unit(3, 3), junk_unit(3)],
                12: [junk_unit(10)],
                13: [y_unit(8, 0), y_unit(8, 1), y_unit(9, 0), y_unit(9, 1),
                     junk_unit(2)],
                14: [y_unit(10, 0), y_unit(10, 1), y_unit(11, 0), y_unit(11, 1),
                     junk_unit(2)],
                15: [junk_unit(7)],
            }
            pending = []

            # ---- attention: flat stream of 256 k-tile steps ----
            rounds = [(qc, hp) for qc in range(SC) for hp in range(NHP)]
            ov_tiles = {}
            ptq = []  # (r, kt, ptile)

            def attn_v(r, kt, ptile):
                qc, hp = rounds[r]
                if r not in ov_tiles:
                    ov_tiles[r] = [
                        ps_ov.tile([128, QCW], DT.float32, tag=f"ov{par}",
                                   name=f"ov{r}_{par}")
                        for par in range(2)
                    ]
                ovs = ov_tiles[r]
                for par in range(2):
                    v0 = (kt * HLOC + 2 * hp + par) * VW
                    w = 128 if v0 + 128 <= ST * HLOC * VW else D + 1
                    nc.tensor.matmul(
                        ovs[par][0:w, :],
                        v_flat[:, v0:v0 + w],
                        ptile[:, par, :],
                        start=(kt == 0),
                        stop=(kt == ST - 1),
                    )

            norm_state = {}

            def norm_a(r):
                # copies only: releases the ov psum banks quickly and
                # keeps the DVE FIFO clear for filler drains
                ovs = ov_tiles.pop(r)
                ovsts = []
                for par in range(2):
                    ovst = norm2p.tile([D, QCW], DT.float32, tag=f"ovst{par}")
                    nc.vector.tensor_copy(ovst, ovs[par][0:D, :])
                    ovsts.append(ovst)
                sums = norm2p.tile([33, QCW], DT.float32, tag="sums")
                for par in range(2):
                    nc.vector.tensor_copy(
                        sums[32 * par:32 * par + 1, :], ovs[par][D:D + 1, :])
                norm_state[r] = (ovsts, sums)

            def norm_b(r):
                qc, hp = rounds[r]
                q0 = qc * QCW
                ovsts, sums = norm_state.pop(r)
                nc.vector.reciprocal(sums, sums)
                for par in range(2):
                    if par == 1:
                        nc.vector.tensor_copy(sums[0:1, :], sums[32:33, :])
                    bc = norm2p.tile([D, QCW], DT.float32, tag="bc")
                    nc.gpsimd.partition_broadcast(bc, sums[0:1, :])
                    nc.vector.tensor_mul(
                        ot_sb[par * D:par * D + D, hp, q0:q0 + QCW],
                        ovsts[par],
                        bc,
                    )

            for i in range(ST * len(rounds)):
                r, kt = divmod(i, ST)
                qc, hp = rounds[r]
                if kt == 0:
                    pending.extend(fillers.get(r, ()))
                sps = ps_s.tile([128, 2, QCW], DT.float32, tag="ps_s")
                for par in range(2):
                    prow = slice(par * D, par * D + D)
                    nc.tensor.matmul(
                        sps[:, par, :],
                        kT[prow, hp, kt * 128:(kt + 1) * 128],
                        qT[prow, hp, qc * QCW:(qc + 1) * QCW],
                        start=True,
                        stop=True,
                    )
                ptile = ptp.tile([128, 2, QCW], DT.bfloat16, tag="pt")
                nc.scalar.activation(
                    out=ptile, in_=sps, func=AF.Exp, scale=float(SCALE),
                )
                ptq.append((r, kt, ptile))
                if len(ptq) > LAG:
                    attn_v(*ptq.pop(0))
                if kt == LAG - 1 and r > 0:
                    norm_a(r - 1)
                if kt == LAG + 3 and r > 0:
                    norm_b(r - 1)
                # drip-feed ~2 filler matmuls into the PE slack
                if pending:
                    try:
                        next(pending[0])
                    except StopIteration:
                        pending.pop(0)

            while ptq:
                attn_v(*ptq.pop(0))
            for g in pending:
                for _ in g:
                    pass
            norm_a(15)
            # keep-warm bridge spanning the final norm chain: emitted
            # BEFORE the Y matmuls (whose weight-loads serialize on the
            # last norm multiply) so the PE queue isn't head-of-line
            # blocked and the HAM clock gate stays warm
            for _ in junk_unit(22):
                pass
            for nch in range(2):
                for _ in y_start(12, nch, 3):
                    pass
            norm_b(15)

            # ---- tail: finish Y of the last q-chunk ----
            y_finish(12, 0, 3)
            y_finish(12, 1, 3)
            for st in range(13, ST):
                for nch in range(2):
                    for _ in y_unit(st, nch):
                        pass

    nc.finalize()
    return nc


def _get_nc():
    with _lock:
        if "nc" not in _cache:
            _cache["nc"] = _build()
        return _cache["nc"]


def _in_maps(q, k, v, Wq, Wk, Wv, Wo):
    import ml_dtypes

    bf16 = ml_dtypes.bfloat16
    xT = {}
    for b in range(B):
        xT[b] = tuple(
            np.ascontiguousarray(t[b].astype(bf16).T) for t in (q, k, v)
        )
    w_bf = [
        (np.ascontiguousarray(W[:, hf * E:(hf + 1) * E].astype(bf16)) if W is not Wo
         else np.ascontiguousarray(W[hf * E:(hf + 1) * E, :].astype(bf16)))
        for hf in range(2) for W in (Wq, Wk, Wv, Wo)
    ]
    maps = []
    for c in range(8):
        b, hf = c // 2, c % 2
        qt, kt, vt = xT[b]
        wqc, wkc, wvc, woc = w_bf[hf * 4:(hf + 1) * 4]
        maps.append({
            "xqT": qt,
            "xkT": kt,
            "xvT": vt,
            "wq": wqc,
            "wk": wkc,
            "wv": wvc,
            "wo": woc,
        })
    return maps


def run(q, k, v, Wq, Wk, Wv, Wo, **spmd_kwargs):
    nc = _get_nc()
    res = run_bass_kernel_spmd(
        nc, _in_maps(q, k, v, Wq, Wk, Wv, Wo), core_ids=list(range(8)),
        **spmd_kwargs,
    )
    out = np.empty((B, S, HID), dtype=np.float32)
    for b in range(B):
        out[b] = res.results[2 * b]["y"] + res.results[2 * b + 1]["y"]
    return out, res


def kernel(q, k, v, Wq, Wk, Wv, Wo):
    out, _ = run(q, k, v, Wq, Wk, Wv, Wo)
    return out


# revision 33
# speedup vs baseline: 1.0159x; 1.0061x over previous
"""Multi-head attention kernel for Trainium2, 8 NeuronCores.

Problem: B=4, S=2048, HID=1024, H=16 heads, D=64.
  Q = q@Wq, K = k@Wk, V = v@Wv (reshaped to heads)
  O = softmax(Q K^T / sqrt(D)) V ;  out = O @ Wo

Sharding (hardcoded): core c handles batch b=c//2 and head-half hf=c%2
(8 of 16 heads via column-parallel Wq/Wk/Wv, row-parallel Wo).  Each core
returns a partial output [S, HID]; the host sums the two head-halves per
batch.

Design (single fused pipeline, ACT-exp paced; ~398us vs 551us baseline):
  - Host pre-transposes q/k/v to [HID, S] and converts x + weights to
    bf16: no on-chip transposes; every projection matmul reads xT with
    the contraction dim on partitions.
  - The softmax exp on the scalar (ACT) engine is the hard floor
    (8 heads x 2048 x 2048 = 33.5M elem/core at ~1.06us per 1024-wide
    psum drain ~= 272us).  The kernel is one flat stream of 256 k-tile
    steps (16 rounds = 4 q-chunks x 4 head-pairs; 16 k-tiles each) of
    [row-packed scores pair -> exp -> attn@V lagging LAG steps], with
    the K/Q/Y projection matmuls drip-fed ~2 per step from a global
    generator deque.  Fillers both hide the projection work inside the
    ACT-paced slack and keep the PE dense enough that the HAM clock
    gate stays at full rate (thin rounds get dead "junk" matmuls).
  - V carries a ones column so attn@V also accumulates the softmax row
    sums; attn@V weight-loads are padded to 128 columns (FWL).
  - DMA: descriptor generation costs ~0.6us per contiguous line on the
    issuing engine queue and transfers cap at ~250-300 GB/s, so x
    tensors stream as one-line chunks interleaved across the sync and
    scalar queues; the head is V proj (DMA-paced) + K et0 + Q qc0.
  - Normalization per round r is split: copies (ovst + packed sums) at
    round r+1 kt LAG-1 release the single-buffered ov psum; reciprocal
    ([33,512], rows 0/32), gpsimd partition-broadcast and multiply at
    kt LAG+3 so the 3.3us reciprocal never blocks filler drains in the
    DVE FIFO.  Y fillers run a full round after their q-chunk's last
    norm (their reversed-AP weight-loads conservatively serialize on
    the most recent norm multiply emitted before them).
  - Tail: junk-matmul bridge spans the final norm chain so the last
    q-chunk's Y projection runs at full clock.
"""

import threading

import numpy as np

import concourse.bacc as bacc
import concourse.mybir as mybir
import concourse.tile as tile
from concourse.bass_utils import run_bass_kernel_spmd

DT = mybir.dt
AF = mybir.ActivationFunctionType

B, S, HID, H = 4, 2048, 1024, 16
D = HID // H               # 64
E = 512                    # local hidden (8 heads)
HLOC = 8                   # heads per core
NHP = 4                    # head pairs per core
SC = 4                     # s-chunks of 512
ST = 16                    # s-tiles of 128
CB = 8                     # contraction blocks of 128 (over HID)
ET = 4                     # e-tiles of 128 in Q^T/K^T
QCW = 512                  # q-chunk width
VW = D + 2                 # V row width: 64 data + ones col + pad (4B align)
LAG = 4                    # attn@V trails scores/exp by this many k-tiles
SCALE = 1.0 / np.sqrt(np.float32(D))   # 0.125

_lock = threading.Lock()
_cache = {}


def _build():
    nc = bacc.Bacc(None)
    xqT = nc.declare_dram_parameter("xqT", [HID, S], DT.bfloat16, isOutput=False)
    xkT = nc.declare_dram_parameter("xkT", [HID, S], DT.bfloat16, isOutput=False)
    xvT = nc.declare_dram_parameter("xvT", [HID, S], DT.bfloat16, isOutput=False)
    wq = nc.declare_dram_parameter("wq", [HID, E], DT.bfloat16, isOutput=False)
    wk = nc.declare_dram_parameter("wk", [HID, E], DT.bfloat16, isOutput=False)
    wv = nc.declare_dram_parameter("wv", [HID, E], DT.bfloat16, isOutput=False)
    wo = nc.declare_dram_parameter("wo", [E, HID], DT.bfloat16, isOutput=False)
    y = nc.declare_dram_parameter("y", [S, HID], DT.float32, isOutput=True)

    with tile.TileContext(nc) as tc:
        with (
            tc.tile_pool(name="wpool", bufs=1) as wpool,
            tc.tile_pool(name="xpool", bufs=1) as xpool,
            tc.tile_pool(name="xqpool", bufs=1) as xqpool,
            tc.tile_pool(name="qkv", bufs=1) as qkvp,
            tc.tile_pool(name="pt", bufs=LAG + 2) as ptp,
            tc.tile_pool(name="norm2", bufs=2) as norm2p,
            tc.tile_pool(name="yout", bufs=2) as youtp,
            tc.tile_pool(name="ps_proj", bufs=2, space="PSUM") as ps_proj,
            tc.tile_pool(name="ps_s", bufs=2, space="PSUM") as ps_s,
            tc.tile_pool(name="ps_ov", bufs=1, space="PSUM") as ps_ov,
        ):
            # ---- DMAs: descriptor-gen spread across engine queues so
            # transfers overlap; x tensors in (cb, s-half) chunks so the
            # V/K projections start as soon as their half has landed.
            wv_sb = wpool.tile([128, CB, E], DT.bfloat16, tag="wv")
            nc.sync.dma_start(
                out=wv_sb, in_=wv.rearrange("(cb p) e -> p cb e", p=128))
            wk_sb = wpool.tile([128, CB, E], DT.bfloat16, tag="wk")
            nc.scalar.dma_start(
                out=wk_sb, in_=wk.rearrange("(cb p) e -> p cb e", p=128))
            xv_sb = xpool.tile([128, CB, S], DT.bfloat16, tag="xv")
            xk_sb = xpool.tile([128, CB, S], DT.bfloat16, tag="xk")
            for h in range(2):
                hs = slice(h * (S // 2), (h + 1) * (S // 2))
                for cb in range(CB):
                    nc.sync.dma_start(
                        out=xv_sb[:, cb, hs],
                        in_=xvT[cb * 128:(cb + 1) * 128, hs])
                for cb in range(CB):
                    nc.scalar.dma_start(
                        out=xk_sb[:, cb, hs],
                        in_=xkT[cb * 128:(cb + 1) * 128, hs])
            wq_sb = wpool.tile([128, CB, E], DT.bfloat16, tag="wq")
            nc.scalar.dma_start(
                out=wq_sb, in_=wq.rearrange("(cb p) e -> p cb e", p=128))

            xq_tiles = {}

            def q_prep(qc, eng=None):
                xq_sb = xqpool.tile([128, CB, QCW], DT.bfloat16, tag="xq",
                                    name=f"xq{qc}")
                (eng or nc.sync).dma_start(
                    out=xq_sb,
                    in_=xqT[:, qc * QCW:(qc + 1) * QCW].rearrange(
                        "(cb p) s -> p cb s", p=128),
                )
                xq_tiles[qc] = xq_sb

            q_prep(0, eng=nc.scalar)
            wo_sb = wpool.tile([128, NHP, HID], DT.bfloat16, tag="wo")
            nc.scalar.dma_start(
                out=wo_sb, in_=wo.rearrange("(eb p) n -> p eb n", p=128))

            # ---- persistent SBUF tensors ----
            qT = qkvp.tile([128, ET, S], DT.bfloat16, tag="qT")
            kT = qkvp.tile([128, ET, S], DT.bfloat16, tag="kT")
            v_sb = qkvp.tile([128, ST, HLOC, VW], DT.bfloat16, tag="v")
            vpad = qkvp.tile([128, 64], DT.bfloat16, tag="vpad")
            _ = vpad
            nc.vector.memset(v_sb[:, :, :, D:D + 1], 1.0)
            v_flat = v_sb.rearrange("p a h w -> p (a h w)")
            ot_sb = qkvp.tile([128, NHP, S], DT.bfloat16, tag="ot")

            # preload the exp table set (~2.7us) during the head
            warm = norm2p.tile([1, 8], DT.float32, tag="warm")
            nc.vector.memset(warm, 0.0)
            nc.scalar.activation(out=warm, in_=warm, func=AF.Exp)

            # ---- projection units (generators: ~2 matmuls per step) ----
            def v_unit(st):
                pp = ps_proj.tile([128, E], DT.float32, tag="pp", name=f"vp{st}")
                for cb in range(CB):
                    nc.tensor.matmul(
                        pp,
                        xv_sb[:, cb, st * 128:(st + 1) * 128],
                        wv_sb[:, cb, :],
                        start=(cb == 0),
                        stop=(cb == CB - 1),
                    )
                nc.vector.tensor_copy(
                    v_sb[:, st, :, 0:D],
                    pp.rearrange("p (h d) -> p h d", h=HLOC),
                )

            def k_unit(et, sc):
                pp = ps_proj.tile([128, QCW], DT.float32, tag="pp",
                                  name=f"kp{et}_{sc}")
                for cb in range(CB):
                    nc.tensor.matmul(
                        pp,
                        wk_sb[:, cb, et * 128:(et + 1) * 128],
                        xk_sb[:, cb, sc * QCW:(sc + 1) * QCW],
                        start=(cb == 0),
                        stop=(cb == CB - 1),
                    )
                    if cb % 2 == 1 and cb < CB - 1:
                        yield
                nc.vector.tensor_copy(kT[:, et, sc * QCW:(sc + 1) * QCW], pp)

            def q_unit(qc, et):
                xq_sb = xq_tiles[qc]
                pp = ps_proj.tile([128, QCW], DT.float32, tag="pp",
                                  name=f"qp{qc}_{et}")
                for cb in range(CB):
                    nc.tensor.matmul(
                        pp,
                        wq_sb[:, cb, et * 128:(et + 1) * 128],
                        xq_sb[:, cb, :],
                        start=(cb == 0),
                        stop=(cb == CB - 1),
                    )
                    if cb % 2 == 1 and cb < CB - 1:
                        yield
                nc.vector.tensor_copy(qT[:, et, qc * QCW:(qc + 1) * QCW], pp)

            y_tiles = {}
            yp_tiles = {}

            def y_start(st, nch, n_hp):
                if nch == 0:
                    y_tiles[st] = youtp.tile(
                        [128, HID], DT.float32, tag="y", name=f"ysb{st}")
                yp = ps_proj.tile([128, QCW], DT.float32, tag="pp",
                                  name=f"yp{st}_{nch}")
                yp_tiles[(st, nch)] = yp
                for hp in range(n_hp):
                    nc.tensor.matmul(
                        yp,
                        ot_sb[:, hp, st * 128:(st + 1) * 128],
                        wo_sb[:, hp, nch * QCW:(nch + 1) * QCW],
                        start=(hp == 0),
                        stop=False,
                    )
                    if hp == 1:
                        yield

            def y_finish(st, nch, hp0):
                yp = yp_tiles.pop((st, nch))
                ysb = y_tiles[st]
                for hp in range(hp0, NHP):
                    nc.tensor.matmul(
                        yp,
                        ot_sb[:, hp, st * 128:(st + 1) * 128],
                        wo_sb[:, hp, nch * QCW:(nch + 1) * QCW],
                        start=False,
                        stop=(hp == NHP - 1),
                    )
                nc.vector.tensor_copy(ysb[:, nch * QCW:(nch + 1) * QCW], yp)
                if nch == 1:
                    nc.sync.dma_start(out=y[st * 128:(st + 1) * 128, :], in_=ysb)

            def y_unit(st, nch):
                yield from y_start(st, nch, 2)
                y_finish(st, nch, 2)

            def y_prefix(st, nch):
                yield from y_start(st, nch, 2)

            def dma_unit(fn):
                fn()
                return
                yield  # generator

            def skip(n):
                for _ in range(n):
                    yield

            junk_n = [0]

            def junk_unit(n):
                # dead score-shaped matmuls: keep the PE dense enough that
                # the HAM clock gate stays at full rate in thin rounds
                for i in range(n):
                    junk_n[0] += 1
                    jp = ps_s.tile([128, 2, QCW], DT.float32, tag="ps_s",
                                   name=f"junk{junk_n[0]}")
                    for par in range(2):
                        nc.tensor.matmul(
                            jp[:, par, :],
                            kT[par * D:par * D + D, 0, 0:128],
                            qT[par * D:par * D + D, 0, 0:QCW],
                            start=True,
                            stop=True,
                        )
                    yield

            # ---- serial head: V, K et0, Q qc0 et0..1 ----
            for st in range(ST):
                v_unit(st)
            for sc in range(SC):
                for _ in k_unit(0, sc):
                    pass
            for et in range(2):
                for _ in q_unit(0, et):
                    pass

            # ---- filler schedule (deque order; ~1 step per k-tile).
            # K(et,sc) before round hp=et reaches k-tile 4*sc (JIT);
            # Q(qc,et) before round 4qc+et; Y(qc,..) with hp3 >=4 k-tiles
            # after norm(qc,hp3) which is emitted at round 4qc+4, kt2.
            fillers = {
                0: [dma_unit(lambda: q_prep(1)), k_unit(1, 0), k_unit(1, 1), k_unit(1, 2)],
                1: [k_unit(1, 3), q_unit(0, 2), k_unit(2, 0), k_unit(2, 1)],
                2: [k_unit(2, 2), k_unit(2, 3), k_unit(3, 0), q_unit(0, 3)],
                3: [k_unit(3, 1), k_unit(3, 2), k_unit(3, 3), q_unit(1, 0)],
                4: [q_unit(1, 1), q_unit(1, 2), q_unit(1, 3),
                    dma_unit(lambda: q_prep(2)), junk_unit(2)],
                5: [y_unit(0, 0), y_unit(0, 1), y_unit(1, 0), q_unit(2, 0)],
                6: [y_unit(1, 1), y_unit(2, 0), y_unit(2, 1), q_unit(2, 1)],
                7: [y_unit(3, 0), y_unit(3, 1), q_unit(2, 2), q_unit(2, 3),
                    dma_unit(lambda: q_prep(3))],
                8: [q_unit(3, 0), junk_unit(6)],
                9: [y_unit(4, 0), y_unit(4, 1), y_unit(5, 0), q_unit(3, 1)],
                10: [y_unit(5, 1), y_unit(6, 0), y_unit(6, 1), q_unit(3, 2)],
                11: [y_unit(7, 0), y_unit(7, 1), q_unit(3, 3), junk_unit(3)],
                12: [junk_unit(10)],
                13: [y_unit(8, 0), y_unit(8, 1), y_unit(9, 0), y_unit(9, 1),
                     junk_unit(2)],
                14: [y_unit(10, 0), y_unit(10, 1), y_unit(11, 0), y_unit(11, 1),
                     junk_unit(2)],
                15: [junk_unit(7)],
            }
            pending = []

            # ---- attention: flat stream of 256 k-tile steps ----
            rounds = [(qc, hp) for qc in range(SC) for hp in range(NHP)]
            ov_tiles = {}
            ptq = []  # (r, kt, ptile)

            def attn_v(r, kt, ptile):
                qc, hp = rounds[r]
                if r not in ov_tiles:
                    ov_tiles[r] = [
                        ps_ov.tile([128, QCW], DT.float32, tag=f"ov{par}",
                                   name=f"ov{r}_{par}")
                        for par in range(2)
                    ]
                ovs = ov_tiles[r]
                for par in range(2):
                    v0 = (kt * HLOC + 2 * hp + par) * VW
                    w = 128 if v0 + 128 <= ST * HLOC * VW else D + 1
                    nc.tensor.matmul(
                        ovs[par][0:w, :],
                        v_flat[:, v0:v0 + w],
                        ptile[:, par, :],
                        start=(kt == 0),
                        stop=(kt == ST - 1),
                    )

            norm_state = {}

            def norm_a(r):
                # copies only: releases the ov psum banks quickly and
                # keeps the DVE FIFO clear for filler drains
                ovs = ov_tiles.pop(r)
                ovsts = []
                for par in range(2):
                    ovst = norm2p.tile([D, QCW], DT.float32, tag=f"ovst{par}")
                    nc.vector.tensor_copy(ovst, ovs[par][0:D, :])
                    ovsts.append(ovst)
                sums = norm2p.tile([33, QCW], DT.float32, tag="sums")
                for par in range(2):
                    nc.vector.tensor_copy(
                        sums[32 * par:32 * par + 1, :], ovs[par][D:D + 1, :])
                norm_state[r] = (ovsts, sums)

            def norm_b(r):
                qc, hp = rounds[r]
                q0 = qc * QCW
                ovsts, sums = norm_state.pop(r)
                nc.vector.reciprocal(sums, sums)
                for par in range(2):
                    if par == 1:
                        nc.vector.tensor_copy(sums[0:1, :], sums[32:33, :])
                    bc = norm2p.tile([D, QCW], DT.float32, tag="bc")
                    nc.gpsimd.partition_broadcast(bc, sums[0:1, :])
                    nc.vector.tensor_mul(
                        ot_sb[par * D:par * D + D, hp, q0:q0 + QCW],
                        ovsts[par],
                        bc,
                    )

            for i in range(ST * len(rounds)):
                r, kt = divmod(i, ST)
                qc, hp = rounds[r]
                if kt == 0:
                    pending.extend(fillers.get(r, ()))
                sps = ps_s.tile([128, 2, QCW], DT.float32, tag="ps_s")
                for par in range(2):
                    prow = slice(par * D, par * D + D)
                    nc.tensor.matmul(
                        sps[:, par, :],
                        kT[prow, hp, kt * 128:(kt + 1) * 128],
                        qT[prow, hp, qc * QCW:(qc + 1) * QCW],
                        start=True,
                        stop=True,
                    )
                ptile = ptp.tile([128, 2, QCW], DT.bfloat16, tag="pt")
                nc.scalar.activation(
                    out=ptile, in_=sps, func=AF.Exp, scale=float(SCALE),
                )
                ptq.append((r, kt, ptile))
                if len(ptq) > LAG:
                    attn_v(*ptq.pop(0))
                if kt == LAG - 1 and r > 0:
                    norm_a(r - 1)
                if kt == LAG + 3 and r > 0:
                    norm_b(r - 1)
                # drip-feed ~2 filler matmuls into the PE slack
                if pending:
                    try:
                        next(pending[0])
                    except StopIteration:
                        pending.pop(0)

            while ptq:
                attn_v(*ptq.pop(0))
            for g in pending:
                for _ in g:
                    pass
            norm_a(15)
            # keep-warm bridge spanning the final norm chain: emitted
            # BEFORE the Y matmuls (whose weight-loads serialize on the
            # last norm multiply) so the PE queue isn't head-of-line
            # blocked and the HAM clock gate stays warm
            for _ in junk_unit(22):
                pass
            for nch in range(2):
                for _ in y_start(12, nch, 3):
                    pass
            norm_b(15)

            # ---- tail: finish Y of the last q-chunk ----
            y_finish(12, 0, 3)
            y_finish(12, 1, 3)
            for st in range(13, ST):
                for nch in range(2):
                    for _ in y_unit(st, nch):
                        pass

    nc.finalize()
    return nc


def _get_nc():
    with _lock:
        if "nc" not in _cache:
            _cache["nc"] = _build()
        return _cache["nc"]


def _in_maps(q, k, v, Wq, Wk, Wv, Wo):
    import ml_dtypes

    bf16 = ml_dtypes.bfloat16
    xT = {}
    for b in range(B):
        xT[b] = tuple(
            np.ascontiguousarray(t[b].astype(bf16).T) for t in (q, k, v)
        )
    w_bf = [
        (np.ascontiguousarray(W[:, hf * E:(hf + 1) * E].astype(bf16)) if W is not Wo
         else np.ascontiguousarray(W[hf * E:(hf + 1) * E, :].astype(bf16)))
        for hf in range(2) for W in (Wq, Wk, Wv, Wo)
    ]
    maps = []
    for c in range(8):
        b, hf = c // 2, c % 2
        qt, kt, vt = xT[b]
        wqc, wkc, wvc, woc = w_bf[hf * 4:(hf + 1) * 4]
        maps.append({
            "xqT": qt,
            "xkT": kt,
            "xvT": vt,
            "wq": wqc,
            "wk": wkc,
            "wv": wvc,
            "wo": woc,
        })
    return maps


def run(q, k, v, Wq, Wk, Wv, Wo, **spmd_kwargs):
    nc = _get_nc()
    res = run_bass_kernel_spmd(
        nc, _in_maps(q, k, v, Wq, Wk, Wv, Wo), core_ids=list(range(8)),
        **spmd_kwargs,
    )
    out = np.empty((B, S, HID), dtype=np.float32)
    for b in range(B):
        out[b] = res.results[2 * b]["y"] + res.results[2 * b + 1]["y"]
    return out, res


def kernel(q, k, v, Wq, Wk, Wv, Wo):
    out, _ = run(q, k, v, Wq, Wk, Wv, Wo)
    return out
